# revision 5
# baseline (speedup 1.0000x reference)
"""Trainium2 Bass kernel for nn_AttnDecoder_87230785782556 (v2).

Multi-head attention decoder: out = softmax((xq Wq)(xk Wk)^T * s) (xv Wv) Wo
Sharding: 8 cores = 2 batches x 4 head-groups (4 heads each).

v2 vs baseline:
- Scores via fp8e4 DoubleRow matmuls (2x f32r score throughput): Q/K
  projections emit PSUM in a (feature-half slot, 32-lane head-major) layout,
  converted to fp8 in SBUF; each score instruction contracts all 64 head
  features at 0.5 cycles/row. The qk scale is folded into the exp
  activation's scale operand so fp8 operands keep their natural range.
- x inputs and Wq/Wk/Wv are bf16 (same PE rate, half the DMA bytes), exp
  outputs and V tiles bf16. Scores dominate the numeric error (fp8).
- x tiles load in [128, 1024] double-chunk DMAs; the wq/xq chain issues
  from the (prologue-idle) Activation DGE so it is not gated behind the
  K/V transfers on the SP queue; a short warm-up spin ramps the PE out of
  its cold p-state while the first DMAs are in flight.
- The activation engine (exp) is the ~133us floor; the schedule keeps it
  fed: q-chunk 0 runs in head-pair phases with the K/V/Q projection train
  interleaved so all four heads' exps overlap projections; later chunks
  run head-outer, with the next chunk's Q projection prefetched mid-chunk
  and the deferred normalization/output-projection of the previous chunk
  dripped piecewise between attention blocks.
- Softmax denominators never leave SBUF: reciprocal in-place on staging
  rows + two rank-1 indicator matmuls broadcast 1/den across partitions.
- PSUM is bank-aware (start=True zeroes a 2KB bank): 3x4KB rotating slots
  + 2x2KB attention accumulators.
- The last chunk's normalization/output projection is split per head-pair
  (pair-0 work overlaps pair-1 attention, pair-1 accumulates into the fin
  tiles with a DVE add) to shorten the drain tail.
"""
import math
import numpy as np

from concourse import bacc, mybir, tile
from concourse.bass_utils import run_bass_kernel_spmd

# Problem constants (hardcoded per contract)
B = 2
SEQ = 2048
E = 1024
NUM_HEADS = 16
HD = 64
QK_SCALE = 0.125
N_CORES = 8
HPC = 4            # heads per core
P = 128
NQ = 512           # q chunk
NH = 256           # q half-window for DoubleRow scores (rhs free = 2*NH)

F32 = mybir.dt.float32
F32R = mybir.dt.float32r
BF16 = mybir.dt.bfloat16
FP8 = mybir.dt.float8e4

EXP_SCALE = float(QK_SCALE / math.sqrt(B))
DR = mybir.MatmulPerfMode.DoubleRow


def build_program(seq=SEQ, repeat=1):
    """Build the per-core SPMD program. Identical on all 8 cores."""
    nc = bacc.Bacc("TRN2", target_bir_lowering=False, debug=False,
                   num_devices=N_CORES)

    n_qc = seq // NQ            # q chunks
    n_kv = seq // P             # kv tiles of 128
    n_g4 = n_kv // 4            # 4-kv-tile score blocks
    n_kt = E // P               # embedding contraction tiles

    xtq = nc.dram_tensor("xtq", [E, seq], BF16, kind="ExternalInput")
    xtk = nc.dram_tensor("xtk", [E, seq], BF16, kind="ExternalInput")
    xtv = nc.dram_tensor("xtv", [E, seq], BF16, kind="ExternalInput")
    wq = nc.dram_tensor("wq", [E, 2, P], BF16, kind="ExternalInput")
    wk = nc.dram_tensor("wk", [E, 2, P], BF16, kind="ExternalInput")
    wv = nc.dram_tensor("wv", [E, HPC * (HD + 1)], BF16, kind="ExternalInput")
    wo = nc.dram_tensor("wo", [HPC * HD, E], F32R, kind="ExternalInput")
    ind = nc.dram_tensor("ind", [P, 2, P], F32R, kind="ExternalInput")
    out = nc.dram_tensor("out", [seq, E], F32, kind="ExternalOutput")

    with tile.TileContext(nc) as tc, nc.allow_low_precision("fp8/bf16 attn"):
        import contextlib
        ctx = contextlib.ExitStack()
        with ctx:
            consts = ctx.enter_context(tc.tile_pool(name="consts", bufs=1))
            khp = ctx.enter_context(tc.tile_pool(name="khp", bufs=1))
            qhp = ctx.enter_context(tc.tile_pool(name="qhp", bufs=2))
            vhp = ctx.enter_context(tc.tile_pool(name="vhp", bufs=n_kv // 2))
            xs = ctx.enter_context(tc.tile_pool(name="xs", bufs=18))
            expp = ctx.enter_context(tc.tile_pool(name="expp", bufs=9))
            stkp = ctx.enter_context(tc.tile_pool(name="stkp", bufs=4))
            finp = ctx.enter_context(tc.tile_pool(name="finp", bufs=2))
            ps = ctx.enter_context(tc.tile_pool(name="ps", bufs=2, space="PSUM"))

            # ---- resident constants (wk/wq eagerly; wv/wo/ind lazily) ----
            wk_t = consts.tile([P, n_kt, 2, P], BF16, name="wk_t", tag="wk")
            wq_t = consts.tile([P, n_kt, 2, P], BF16, name="wq_t", tag="wq")
            wv_t = consts.tile([P, n_kt, HPC * (HD + 1)], BF16, name="wv_t", tag="wv")
            wo_t = consts.tile([P, 2, E], F32R, name="wo_t", tag="wo")
            ind_t = consts.tile([P, 2, P], F32R, name="ind_t", tag="ind")
            nc.sync.dma_start(out=wk_t, in_=wk.ap().rearrange("(t p) s m -> p t s m", p=P))

            for rep in range(repeat):
                kh8 = khp.tile([P, 2, seq], FP8, name="kh8", tag="kh8")
                vh_tiles = [vhp.tile([P, 2, HPC, HD + 1], BF16, name=f"vh{g}",
                                     tag="vh") for g in range(n_kv // 2)]
                qh8_list = [None] * n_qc
                xk_cache = {}
                xv_cache = {}
                xq_cache = {}

                def emit_kloads(kc2):
                    nh2 = min(2, n_qc - 2 * kc2)
                    for kt in range(n_kt):
                        t = xs.tile([P, 2 * NQ], BF16, name="xk_t", tag="xk",
                                    bufs=16)
                        nc.sync.dma_start(
                            out=t[:, 0:NQ * nh2],
                            in_=xtk.ap()[P * kt:P * (kt + 1),
                                         2 * NQ * kc2:2 * NQ * kc2 + NQ * nh2])
                        xk_cache[kc2, kt] = t

                def emit_kproj(kc):
                    half = kc % 2
                    ps_k = ps.tile([P, 2, NQ], F32, name="ps_k", tag="ps")
                    for kt in range(n_kt):
                        xk_t = xk_cache[kc // 2, kt][:, NQ * half:NQ * (half + 1)]
                        for s in range(2):
                            nc.tensor.matmul(
                                ps_k[:, s, :], wk_t[:, kt, s, :], xk_t,
                                start=(kt == 0), stop=(kt == n_kt - 1))
                    nc.vector.tensor_copy(kh8[:, :, NQ * kc:NQ * (kc + 1)],
                                          ps_k)

                def emit_vchunk1(g):
                    # one kv pair (256 kv positions) of V projection
                    if g % 4 == 0:
                        vw = min(4 * NH, seq - 2 * P * g)
                        for kt in range(n_kt):
                            t = xs.tile([P, 2 * NQ], BF16, name="xv_t", tag="xv",
                                        bufs=9)
                            nc.sync.dma_start(
                                out=t[:, 0:vw],
                                in_=xtv.ap()[P * kt:P * (kt + 1),
                                             2 * P * g:2 * P * g + vw])
                            xv_cache[kt] = t
                    off = 2 * P * (g % 4)
                    ps_v = ps.tile([P, 2, NQ], F32, name="ps_v", tag="ps")
                    for kt in range(n_kt):
                        for u in range(2):
                            nc.tensor.matmul(
                                ps_v[:, u, 0:HPC * (HD + 1)],
                                xv_cache[kt][:, off + P * u:off + P * (u + 1)],
                                wv_t[:, kt, :],
                                start=(kt == 0), stop=(kt == n_kt - 1))
                    vh_t = vh_tiles[g]
                    nc.vector.tensor_copy(
                        vh_t,
                        ps_v[:, :, 0:HPC * (HD + 1)].rearrange(
                            "p s (h c) -> p s h c", h=HPC))
                    # ones slots (zero in wv): even heads col 64, odd col 0
                    nc.vector.tensor_scalar_add(
                        vh_t[:, :, 0::2, HD], vh_t[:, :, 0::2, HD], 1.0)
                    nc.vector.tensor_scalar_add(
                        vh_t[:, :, 1::2, 0], vh_t[:, :, 1::2, 0], 1.0)

                def emit_qproj(qc):
                    if qc % 2 == 0:
                        # q-chunk 0 loads ride the idle ACT DGE so the Q chain
                        # is not gated behind the K/V transfers on the SP queue
                        eng = nc.scalar if qc == 0 else nc.sync
                        qw = min(2 * NQ, seq - NQ * qc)
                        for kt in range(n_kt):
                            t = xs.tile([P, 2 * NQ], BF16, name="xq_t", tag="xq",
                                        bufs=8)
                            eng.dma_start(
                                out=t[:, 0:qw],
                                in_=xtq.ap()[P * kt:P * (kt + 1),
                                             NQ * qc:NQ * qc + qw])
                            xq_cache[kt] = t
                    off = NQ * (qc % 2)
                    qh8_t = qhp.tile([P, 2, NQ], FP8, name="qh8_t", tag="qh8")
                    for s in range(2):
                        ps_q = ps.tile([P, NQ], F32, name="ps_q", tag="opbc",
                                       bufs=2)
                        for kt in range(n_kt):
                            nc.tensor.matmul(
                                ps_q, wq_t[:, kt, s, :],
                                xq_cache[kt][:, off:off + NQ],
                                start=(kt == 0), stop=(kt == n_kt - 1))
                        nc.vector.tensor_copy(qh8_t[:, s, :], ps_q)
                    qh8_list[qc] = qh8_t

                def norm_pair(stk_t, sodd_t, dstg_t):
                    nc.vector.reciprocal(dstg_t[HD:HD + 1, :],
                                         dstg_t[HD:HD + 1, :])
                    nc.vector.reciprocal(sodd_t[0:1, :], sodd_t[0:1, :])
                    bc_ps = ps.tile([P, NQ], F32, name="bc_ps", tag="opbc",
                                    bufs=2)
                    nc.tensor.matmul(bc_ps, ind_t[HD:HD + 1, 0, :],
                                     dstg_t[HD:HD + 1, :],
                                     start=True, stop=False,
                                     skip_group_check=True)
                    nc.tensor.matmul(bc_ps, ind_t[0:1, 1, :], sodd_t[0:1, :],
                                     start=False, stop=True,
                                     skip_group_check=True)
                    nc.vector.tensor_mul(stk_t, stk_t, bc_ps)

                def emit_block(att_t, h, g2, qc, pend_av):
                    # one kv pair x full 512-q window: 2 DoubleRow score
                    # matmuls (1024-element fp8 moving) + one exp
                    sc_t = ps.tile([P, 2, NQ], F32, name="sc_t", tag="ps")
                    for j in range(2):
                        kv = 2 * g2 + j
                        nc.tensor.matmul(
                            sc_t[:, j, :],
                            kh8[32 * h:32 * (h + 1), :, P * kv:P * (kv + 1)],
                            qh8_list[qc][32 * h:32 * (h + 1), :, :],
                            start=True, stop=True,
                            perf_mode=DR,
                            tile_position=(32 * h, 0),
                            skip_group_check=True)
                    e_t = expp.tile([P, 2, NQ], BF16, name="e_t", tag="exp")
                    nc.scalar.activation(e_t, sc_t,
                                         mybir.ActivationFunctionType.Exp,
                                         scale=EXP_SCALE)
                    if pend_av is not None:
                        emit_attnv(*pend_av)
                    return (att_t, h, g2, e_t)

                def emit_attnv(att_t, h, g2, e_t):
                    for j in range(2):
                        kv = 2 * g2 + j
                        nc.tensor.matmul(
                            att_t,
                            vh_tiles[g2][:, j, h, :],
                            e_t[:, j, :],
                            start=(g2 == 0 and j == 0),
                            stop=(g2 == n_kv // 2 - 1 and j == 1),
                            skip_group_check=True)

                def evict_pair(att_a, att_b):
                    # stack the pair's features into stk, stage denominators
                    stk_t = stkp.tile([P, NQ], F32R, name="stk_t", tag="stk")
                    nc.vector.tensor_copy(stk_t[0:HD, :], att_a[0:HD, :])
                    sodd_t = stkp.tile([P, NQ], F32R, name="sodd_t",
                                       tag="sodd", bufs=4)
                    nc.vector.tensor_copy(sodd_t[0:HD + 1, :],
                                          att_b[0:HD + 1, :])
                    dstg_t = stkp.tile([P, NQ], F32R, name="dstg_t",
                                       tag="dstg", bufs=4)
                    nc.vector.tensor_copy(dstg_t[HD:HD + 1, :],
                                          att_a[HD:HD + 1, :])
                    nc.sync.dma_start(out=stk_t[HD:P, :],
                                      in_=sodd_t[1:HD + 1, :])
                    return stk_t, sodd_t, dstg_t

                def emit_outproj_half(stk_t, pair, fin_tiles, qc):
                    # one pair's contribution to the output projection; pair 0
                    # fills fin, pair 1 adds into it and DMAs out
                    for qs in range(NQ // P):
                        fin_t = fin_tiles[qs // 2]
                        a = qs % 2
                        for nch in range(2):
                            op_ps = ps.tile([P, NQ], F32, name="op_ps",
                                            tag="opbc", bufs=2)
                            nc.tensor.matmul(
                                op_ps, stk_t[:, P * qs:P * (qs + 1)],
                                wo_t[:, pair, NQ * nch:NQ * (nch + 1)],
                                start=True, stop=True)
                            if pair == 0:
                                nc.vector.tensor_copy(fin_t[:, a, nch, :], op_ps)
                            else:
                                nc.vector.tensor_add(
                                    fin_t[:, a, nch, :], fin_t[:, a, nch, :],
                                    op_ps)
                                r0 = NQ * qc + P * qs
                                nc.sync.dma_start(
                                    out=out.ap()[r0:r0 + P,
                                                 NQ * nch:NQ * (nch + 1)],
                                    in_=fin_t[:, a, nch, :])

                def make_norm_gen(stk_tiles, sodds, dstgs, qc):
                    def gen():
                        for pair in range(2):
                            norm_pair(stk_tiles[pair], sodds[pair], dstgs[pair])
                        yield
                        for qs2 in range(NQ // P // 2):
                            fin_t = finp.tile([P, 2, 2, NQ], F32, name="fin_t",
                                              tag="fin")
                            for a in range(2):
                                qs = 2 * qs2 + a
                                for nch in range(2):
                                    op_ps = ps.tile([P, NQ], F32, name="op_ps",
                                                    tag="opbc", bufs=2)
                                    for pair in range(2):
                                        nc.tensor.matmul(
                                            op_ps,
                                            stk_tiles[pair][:, P * qs:P * (qs + 1)],
                                            wo_t[:, pair, NQ * nch:NQ * (nch + 1)],
                                            start=(pair == 0), stop=(pair == 1))
                                    nc.vector.tensor_copy(fin_t[:, a, nch, :],
                                                          op_ps)
                                r0 = NQ * qc + 2 * P * qs2 + P * a
                                nc.sync.dma_start(
                                    out=out.ap()[r0:r0 + P, :],
                                    in_=fin_t[:, a].rearrange("p n q -> p (n q)"))
                            yield
                    return gen()

                pending = None
                for qc in range(n_qc):
                    last_qc = qc == n_qc - 1
                    norm_gen = (make_norm_gen(*pending) if pending is not None
                                else None)
                    pending = None
                    stk_tiles = []
                    sodds = []
                    dstgs = []

                    if qc == 0:
                        # head-pair phases with the projection train woven in
                        pend_av = None
                        fin_last = None
                        vnext = 0
                        for hp in range(HPC // 2):
                            heads = (2 * hp, 2 * hp + 1)
                            att_pair = {
                                h: ps.tile([HD + 1, NQ], F32, name="att_t",
                                           tag="att", bufs=2)
                                for h in heads}
                            for g2 in range(n_kv // 2):
                                if hp == 0:
                                    if g2 == 0:
                                        emit_kloads(0)
                                        if rep == 0:
                                            nc.scalar.dma_start(
                                                out=wq_t,
                                                in_=wq.ap().rearrange(
                                                    "(t p) s m -> p t s m", p=P))
                                            # warm the PE clock out of its
                                            # cold p-state while the weight
                                            # and x DMAs are still in flight
                                            warm = xs.tile(
                                                [P, NQ], BF16, name="warm_t",
                                                tag="warm", bufs=1)
                                            nc.gpsimd.memset(warm, 0.0)
                                            ps_w = ps.tile(
                                                [P, NQ], F32, name="ps_w",
                                                tag="opbc", bufs=2)
                                            for _spin in range(10):
                                                nc.tensor.matmul(
                                                    ps_w, warm[:, 0:P], warm,
                                                    start=True, stop=True,
                                                    skip_group_check=True)
                                        if n_qc > 2:
                                            emit_kloads(1)
                                        emit_kproj(0)
                                        if rep == 0:
                                            nc.sync.dma_start(
                                                out=wv_t,
                                                in_=wv.ap().rearrange(
                                                    "(t p) m -> p t m", p=P))
                                        emit_qproj(0)
                                    if (g2 == 2 or n_kv // 2 <= 2) and rep == 0:
                                        nc.sync.dma_start(
                                            out=wo_t,
                                            in_=wo.ap().rearrange(
                                                "(t p) m -> p t m", p=P))
                                        nc.sync.dma_start(out=ind_t,
                                                          in_=ind.ap())
                                    if g2 % 2 == 0 and 1 <= g2 // 2 < n_qc:
                                        emit_kproj(g2 // 2)
                                else:
                                    if g2 == min(2, n_kv // 2 - 1):
                                        if not last_qc:
                                            # prefetch next chunk's Q proj in
                                            # the ACT-bound second phase
                                            emit_qproj(1)
                                        else:
                                            norm_pair(stk_tiles[0], sodds[0],
                                                      dstgs[0])
                                            fin_last = [
                                                finp.tile([P, 2, 2, NQ], F32,
                                                          name="fin_t",
                                                          tag="fin")
                                                for _ in range(2)]
                                            emit_outproj_half(stk_tiles[0], 0,
                                                              fin_last, qc)
                                for h in heads:
                                    if hp == 0 and vnext < n_kv // 2:
                                        # one V pair per block keeps pairs a
                                        # step ahead of the lagging attnV
                                        # consumer stream
                                        emit_vchunk1(vnext)
                                        vnext += 1
                                    pend_av = emit_block(
                                        att_pair[h], h, g2, qc, pend_av)
                            # flush the attnV pipeline, evict this pair
                            if pend_av is not None:
                                emit_attnv(*pend_av)
                                pend_av = None
                            stk_t, sodd_t, dstg_t = evict_pair(
                                att_pair[heads[0]], att_pair[heads[1]])
                            stk_tiles.append(stk_t)
                            sodds.append(sodd_t)
                            dstgs.append(dstg_t)
                        if last_qc:
                            norm_pair(stk_tiles[1], sodds[1], dstgs[1])
                            emit_outproj_half(stk_tiles[1], 1, fin_last, qc)
                    else:
                        fin_last = None
                        att_list = []
                        for h in range(HPC):
                            att_t = ps.tile([HD + 1, NQ], F32, name="att_t",
                                            tag="att", bufs=2)
                            att_list.append(att_t)
                            pend_av = None
                            for g2 in range(n_kv // 2):
                                if (norm_gen is not None and h == 0
                                        and g2 >= 2 and g2 % 2 == 0):
                                    next(norm_gen, None)
                                pend_av = emit_block(att_t, h, g2, qc, pend_av)
                            emit_attnv(*pend_av)

                            if h == 0 and norm_gen is not None:
                                for _ in norm_gen:
                                    pass
                                norm_gen = None

                            if h % 2 == 1:
                                stk_t, sodd_t, dstg_t = evict_pair(
                                    att_list[h - 1], att_list[h])
                                stk_tiles.append(stk_t)
                                sodds.append(sodd_t)
                                dstgs.append(dstg_t)
                                if h == 1:
                                    if not last_qc:
                                        # prefetch next chunk's Q projection
                                        emit_qproj(qc + 1)
                                    else:
                                        norm_pair(stk_tiles[0], sodds[0],
                                                  dstgs[0])
                                        fin_last = [
                                            finp.tile([P, 2, 2, NQ], F32,
                                                      name="fin_t", tag="fin")
                                            for _ in range(2)]
                                        emit_outproj_half(stk_tiles[0], 0,
                                                          fin_last, qc)
                                elif h == 3 and last_qc:
                                    norm_pair(stk_tiles[1], sodds[1], dstgs[1])
                                    emit_outproj_half(stk_tiles[1], 1,
                                                      fin_last, qc)

                    if not last_qc:
                        pending = (stk_tiles, sodds, dstgs, qc)
    nc.finalize()
    return nc


_PROG_CACHE = {}


def _get_program(seq=SEQ, repeat=1):
    key = (seq, repeat)
    if key not in _PROG_CACHE:
        _PROG_CACHE[key] = build_program(seq, repeat)
    return _PROG_CACHE[key]


def shard_inputs(q, k, v, Wq, Wk, Wv, Wo, seq=SEQ):
    """Build the 8 per-core input maps (host-side layout prep)."""
    import ml_dtypes
    bf16 = ml_dtypes.bfloat16
    in_maps = []
    for c in range(N_CORES):
        b = c // 4
        hg = c % 4
        heads = [4 * hg + j for j in range(HPC)]
        # qk projection weights: out partition m (head-major, 32 lanes per
        # head), slot s holds features 32s..32s+31
        col_idx = np.array(
            [[(32 * s + m % 32) * NUM_HEADS + heads[m // 32] for m in range(P)]
             for s in range(2)])
        wq_s = Wq[:, col_idx.reshape(-1)].reshape(E, 2, P)
        wk_s = Wk[:, col_idx.reshape(-1)].reshape(E, 2, P)
        wv_s = np.zeros((E, HPC, HD + 1), dtype=np.float32)
        for j, h in enumerate(heads):
            if j % 2 == 0:
                wv_s[:, j, 0:HD] = Wv[:, h::NUM_HEADS]
            else:
                wv_s[:, j, 1:HD + 1] = Wv[:, h::NUM_HEADS]
        wo_s = np.concatenate([Wo[h::NUM_HEADS, :] for h in heads], axis=0)
        # rank-1 denominator broadcast indicators: row HD slot 0 -> even-head
        # halves (partitions 0..63), row 0 slot 1 -> odd halves (64..127)
        ind = np.zeros((P, 2, P), dtype=np.float32)
        ind[HD, 0, 0:HD] = 1.0
        ind[0, 1, HD:P] = 1.0
        in_maps.append({
            "xtq": np.ascontiguousarray(q[b][:seq].T.astype(bf16)),
            "xtk": np.ascontiguousarray(k[b][:seq].T.astype(bf16)),
            "xtv": np.ascontiguousarray(v[b][:seq].T.astype(bf16)),
            "wq": np.ascontiguousarray(wq_s.astype(bf16)),
            "wk": np.ascontiguousarray(wk_s.astype(bf16)),
            "wv": np.ascontiguousarray(
                wv_s.reshape(E, HPC * (HD + 1)).astype(bf16)),
            "wo": np.ascontiguousarray(wo_s),
            "ind": ind,
        })
    return in_maps


def unshard(results, seq=SEQ):
    out = np.zeros((B, seq, E), dtype=np.float32)
    for c in range(N_CORES):
        out[c // 4] += results[c]["out"]
    return out


def kernel(q, k, v, Wq, Wk, Wv, Wo):
    q = np.asarray(q, dtype=np.float32)
    k = np.asarray(k, dtype=np.float32)
    v = np.asarray(v, dtype=np.float32)
    Wq = np.asarray(Wq, dtype=np.float32)
    Wk = np.asarray(Wk, dtype=np.float32)
    Wv = np.asarray(Wv, dtype=np.float32)
    Wo = np.asarray(Wo, dtype=np.float32)
    nc = _get_program()
    in_maps = shard_inputs(q, k, v, Wq, Wk, Wv, Wo)
    res = run_bass_kernel_spmd(nc, in_maps, list(range(N_CORES)))
    return unshard(res.results)
